# revision 1
# baseline (speedup 1.0000x reference)
"""GAT (5-layer, dense-adjacency) Trainium2 kernel, sharded across 8 NeuronCores.

Sharding: query-node rows split 512/core. Each core holds its transposed
additive attention mask [4096(j), 512(i)] resident in SBUF, computes the
full Wh per layer (cheap), its row-block of attention via a fused custom
DVE op (leaky(s+d)+addmask) + ACT exp + bf16 matmul with a ones-column for
softmax denominators, then AllGathers the transposed activations.
"""

import numpy as np

import concourse.bacc as bacc
import concourse.mybir as mybir
import concourse.tile as tile
from concourse.bass_utils import run_bass_kernel_spmd

import concourse.dve_ops as dve_ops
from concourse.dve_spec import Spec, Src0, Src1, C0, C1, maxx, lower
from concourse.dve_spec import _has_src1 as _spec_has_src1
from concourse.dve_uop import DveOpSpec

try:
    import ml_dtypes

    _BF16 = ml_dtypes.bfloat16
except ImportError:  # pragma: no cover
    _BF16 = np.float32

dt = mybir.dt
AF = mybir.ActivationFunctionType

# ---------------------------------------------------------------- constants
N = 4096
NCORE = 8
ROWS = N // NCORE  # 512 query rows per core
P = 128
JT = N // P  # 32 j-tiles
NEG = -30000.0  # additive mask for non-edges; exp(x-30000) == 0 in f32
ALPHA = 0.1
# (fin, fout, heads, concat, elu_after)
CFG = [
    (256, 128, 8, True, True),
    (128, 64, 8, True, True),
    (64, 32, 4, True, True),
    (32, 16, 1, True, False),
    (16, 8, 1, False, False),
]

# ---------------------------------------------------------------- custom op
LEAKY_BIAS_ADDMASK = dve_ops.DveOp(
    "LEAKY_BIAS_ADDMASK",
    Spec(
        body=maxx(Src0 + C0, (Src0 + C0) * C1) + Src1,
        reference=lambda in0, in1, s0, s1, imm2: (
            np.maximum(in0 + s0, (in0 + s0) * s1) + in1
        ).astype(np.float32),
    ),
    subdim=False,
    uops_sha={},
)


def _register_custom_op(op):
    if op.name in dve_ops._SUB_OPCODE_FOR_NAME:
        return
    idx = dve_ops._CUSTOM_DVE_ROW_BASE + len(dve_ops.OPS)
    assert idx < 0x20
    dve_ops.OPS.append(op)
    dve_ops.CUSTOM_DVE_SPECS[op.name] = op.spec
    dve_ops._SUB_OPCODE_FOR_NAME[op.name] = idx
    shas = {}
    for ver in ("v3", "v4"):
        try:
            s = DveOpSpec(
                name=op.name,
                opcode=idx,
                uops=lower(op.spec, ver=ver),
                rd1_en=_spec_has_src1(op.spec),
            )
            shas[ver] = s.sha(ver)
        except Exception:
            pass
    object.__setattr__(op, "uops_sha", shas)


_register_custom_op(LEAKY_BIAS_ADDMASK)


# ---------------------------------------------------------------- builder
def build_kernel():
    import os as _os
    debug_taps = bool(_os.environ.get("DEBUG_TAPS"))
    nc = bacc.Bacc("TRN2", target_bir_lowering=False, debug=False)

    adjrows = nc.dram_tensor("adjrows", [ROWS, N], dt.int32, kind="ExternalInput")
    x0T_own = nc.dram_tensor("x0T_own", [256, ROWS], dt.float32, kind="ExternalInput")
    wext_dram = {}
    ws_dram = {}
    for li, (fin, fout, h, concat, _elu) in enumerate(CFG, start=1):
        dh = fout // h if concat else fout
        wext_dram[li] = nc.dram_tensor(
            f"wext{li}", [fin, h * dh + h], dt.float32, kind="ExternalInput"
        )
        ws_dram[li] = nc.dram_tensor(f"ws{li}", [fin, h], dt.float32, kind="ExternalInput")

    pool_out = nc.dram_tensor("pool_part", [8, 1], dt.float32, kind="ExternalOutput")
    if debug_taps:
        dbg_d = nc.dram_tensor("dbg_d", [P, JT, 8], dt.float32, kind="ExternalOutput")
        dbg_wh = nc.dram_tensor("dbg_wh", [P, 8 * 33], dt.float32, kind="ExternalOutput")
        dbg_x = {}
        for _li, (_f, _fo, _h, _c, _e) in enumerate(CFG, start=1):
            _fo2 = _fo if _c else _fo
            dbg_x[_li] = nc.dram_tensor(f"dbg_x{_li}", [_fo2, ROWS], dt.float32, kind="ExternalOutput")

    ident_np = np.eye(P, dtype=_BF16)
    ident_dram = nc.inline_tensor(ident_np, name="ident128")

    with tile.TileContext(nc) as tc:
        with (
            tc.tile_pool(name="persist", bufs=1) as persist,
            tc.tile_pool(name="dram", bufs=1, space="DRAM") as drampool,
            tc.tile_pool(name="xTown", bufs=3) as xTown_pool,
            tc.tile_pool(name="layerbuf", bufs=1) as layerbuf,
            tc.tile_pool(name="work", bufs=2) as work,
            tc.tile_pool(name="srep", bufs=4) as srep_pool,
            tc.tile_pool(name="small", bufs=2) as small,
            tc.tile_pool(name="whps", bufs=2, space="PSUM") as whps,
            tc.tile_pool(name="sps", bufs=1, space="PSUM") as sps,
            tc.tile_pool(name="attps", bufs=4, space="PSUM") as attps,
        ):
            # ---------------- persistent tiles
            maskT = persist.tile([P, JT, ROWS], dt.float32, tag="maskT")
            ident_sb = persist.tile([P, P], dt.bfloat16, tag="ident")
            nc.sync.dma_start(ident_sb[:], ident_dram[:])
            ones_row = persist.tile([1, P], dt.float32, tag="ones_row")
            nc.vector.memset(ones_row[:], 1.0)
            negb = persist.tile([P, 1], dt.float32, tag="negb")
            nc.vector.memset(negb[:], NEG)

            wext_sb = {}
            ws_sb = {}
            for li, (fin, fout, h, concat, _elu) in enumerate(CFG, start=1):
                dh = fout // h if concat else fout
                nft = (fin + P - 1) // P
                wext_sb[li] = []
                ws_sb[li] = []
                for ft in range(nft):
                    fr = min(P, fin - ft * P)
                    wt = persist.tile([fr, h * dh + h], dt.float32, tag=f"wext{li}_{ft}")
                    nc.sync.dma_start(wt[:], wext_dram[li][ft * P : ft * P + fr, :])
                    wext_sb[li].append(wt)
                    st = persist.tile([fr, h], dt.float32, tag=f"ws{li}_{ft}")
                    nc.sync.dma_start(st[:], ws_dram[li][ft * P : ft * P + fr, :])
                    ws_sb[li].append(st)

            # ---------------- mask build (transpose adj rows -> additive maskT)
            CH = 1024
            for c0 in range(0, N, CH):
                for ib in range(ROWS // P):
                    stage_i = work.tile([P, CH], dt.int32, tag="stage_i")
                    nc.sync.dma_start(
                        stage_i[:], adjrows[ib * P : (ib + 1) * P, c0 : c0 + CH]
                    )
                    stage_b = work.tile([P, CH], dt.bfloat16, tag="stage_b")
                    nc.gpsimd.tensor_copy(stage_b[:], stage_i[:])
                    for k in range(CH // P):
                        jt = (c0 + k * P) // P
                        tps = sps.tile([P, P], dt.bfloat16, tag="ps_row")
                        nc.tensor.transpose(
                            tps[:], stage_b[:, k * P : (k + 1) * P], ident_sb[:]
                        )
                        nc.scalar.activation(
                            maskT[:, jt, ib * P : (ib + 1) * P],
                            tps[:],
                            AF.Identity,
                            bias=negb[:],
                            scale=-NEG,
                        )

            # ---------------- L1 own activations from input
            xTown_cur = []
            for ft in range(2):
                to = xTown_pool.tile([P, ROWS], dt.float32, tag="xTown")
                nc.sync.dma_start(to[:], x0T_own[ft * P : (ft + 1) * P, :])
                xTown_cur.append(to)

            for li, (fin, fout, h, concat, elu) in enumerate(CFG, start=1):
                dh = fout // h if concat else fout
                hdh = h * dh
                CW = hdh + h  # compact row width: Wh values + d column
                nft = (fin + P - 1) // P
                is_last = li == len(CFG)

                # ---- (A) own-block Wh (+d) for the 4 own j-chunks
                own_hi = work.tile([P, 4, CW], dt.bfloat16, tag="own_hi")
                own_lo = work.tile([P, 4, CW], dt.bfloat16, tag="own_lo")
                for k in range(4):
                    pw = whps.tile([P, CW], dt.float32, tag="pw")
                    for ft in range(nft):
                        fr = min(P, fin - ft * P)
                        nc.tensor.matmul(
                            pw[:],
                            xTown_cur[ft][0:fr, k * P : (k + 1) * P],
                            wext_sb[li][ft][:],
                            start=(ft == 0),
                            stop=(ft == nft - 1),
                        )
                    nc.scalar.copy(own_hi[:, k, :], pw[:])
                    nc.vector.tensor_sub(own_lo[:, k, :], pw[:], own_hi[:, k, :])

                # ---- (B) s_rep per head (from own activations)
                sreps = []
                for hh in range(h):
                    ps_row = sps.tile([1, ROWS], dt.float32, tag="ps_row")
                    for ft in range(nft):
                        fr = min(P, fin - ft * P)
                        nc.tensor.matmul(
                            ps_row[:],
                            ws_sb[li][ft][:, hh : hh + 1],
                            xTown_cur[ft][0:fr, :],
                            start=(ft == 0),
                            stop=(ft == nft - 1),
                        )
                    s_row = small.tile([1, ROWS], dt.float32, tag="vec1")
                    nc.vector.tensor_copy(s_row[:], ps_row[:])
                    ps_rep = sps.tile([P, ROWS], dt.float32, tag="ps_rep")
                    nc.tensor.matmul(
                        ps_rep[:], ones_row[:], s_row[:], start=True, stop=True
                    )
                    srt = srep_pool.tile([P, ROWS], dt.float32, tag="srep")
                    nc.scalar.copy(srt[:], ps_rep[:])
                    sreps.append(srt)

                # ---- (C) pack + AllGather: A = headsA values + d, B = headsB
                hA = (h // 2 if h > 1 else 1) if not _os.environ.get("NOSPLIT") else h
                hB = h - hA
                CWa = hA * dh + h
                CWb = hB * dh
                ag_a_in = drampool.tile([2, 4 * P, CWa], dt.bfloat16, tag=f"again{li}")
                ag_a_out = drampool.tile(
                    [NCORE, 2, 4 * P, CWa], dt.bfloat16, tag=f"agaout{li}"
                )
                nc.sync.dma_start(
                    ag_a_in[0].rearrange("(k p) c -> p k c", p=P), own_hi[:, :, 0:CWa]
                )
                nc.sync.dma_start(
                    ag_a_in[1].rearrange("(k p) c -> p k c", p=P), own_lo[:, :, 0:CWa]
                )
                nc.gpsimd.collective_compute(
                    "AllGather",
                    mybir.AluOpType.bypass,
                    replica_groups=[list(range(NCORE))],
                    ins=[ag_a_in.opt()],
                    outs=[ag_a_out.opt()],
                )
                if hB:
                    ag_b_in = drampool.tile(
                        [2, 4 * P, CWb], dt.bfloat16, tag=f"agbin{li}"
                    )
                    ag_b_out = drampool.tile(
                        [NCORE, 2, 4 * P, CWb], dt.bfloat16, tag=f"agbout{li}"
                    )
                    nc.sync.dma_start(
                        ag_b_in[0].rearrange("(k p) c -> p k c", p=P),
                        own_hi[:, :, CWa:CW],
                    )
                    nc.sync.dma_start(
                        ag_b_in[1].rearrange("(k p) c -> p k c", p=P),
                        own_lo[:, :, CWa:CW],
                    )
                    nc.gpsimd.collective_compute(
                        "AllGather",
                        mybir.AluOpType.bypass,
                        replica_groups=[list(range(NCORE))],
                        ins=[ag_b_in.opt()],
                        outs=[ag_b_out.opt()],
                    )

                # ---- (D/E) load + unpack into padded matmul layout
                whrow = layerbuf.tile([P, JT, h * 33], dt.bfloat16, tag="whrow")
                whrow_lo = layerbuf.tile([P, JT, h * 33], dt.bfloat16, tag="whrow_lo")
                d_sb = layerbuf.tile([P, JT, h], dt.float32, tag="d_sb")
                wh4 = whrow.rearrange("p j (a b) -> p a j b", a=h)
                wl4 = whrow_lo.rearrange("p j (a b) -> p a j b", a=h)
                for hh in range(h):
                    nc.gpsimd.memset(wh4[:, hh, :, dh:33], 0.0)
                    nc.gpsimd.memset(wl4[:, hh, :, dh:33], 0.0)
                    nc.gpsimd.memset(wh4[:, hh, :, 32:33], 1.0)

                cmp_a_hi = layerbuf.tile([P, JT, CWa], dt.bfloat16, tag="cmp_a_hi")
                cmp_a_lo = layerbuf.tile([P, JT, CWa], dt.bfloat16, tag="cmp_a_lo")
                for r in range(NCORE):
                    nc.sync.dma_start(
                        cmp_a_hi[:, 4 * r : 4 * (r + 1), :],
                        ag_a_out[r, 0].rearrange("(k p) c -> p k c", p=P),
                    )
                    nc.sync.dma_start(
                        cmp_a_lo[:, 4 * r : 4 * (r + 1), :],
                        ag_a_out[r, 1].rearrange("(k p) c -> p k c", p=P),
                    )
                nc.vector.tensor_add(
                    d_sb[:], cmp_a_hi[:, :, hA * dh : CWa], cmp_a_lo[:, :, hA * dh : CWa]
                )
                for hh in range(hA):
                    nc.gpsimd.tensor_copy(
                        wh4[:, hh, :, 0:dh], cmp_a_hi[:, :, hh * dh : (hh + 1) * dh]
                    )
                    nc.vector.tensor_copy(
                        wl4[:, hh, :, 0:dh], cmp_a_lo[:, :, hh * dh : (hh + 1) * dh]
                    )
                if hB:
                    cmp_b_hi = layerbuf.tile([P, JT, CWb], dt.bfloat16, tag="cmp_b_hi")
                    cmp_b_lo = layerbuf.tile([P, JT, CWb], dt.bfloat16, tag="cmp_b_lo")
                    for r in range(NCORE):
                        nc.sync.dma_start(
                            cmp_b_hi[:, 4 * r : 4 * (r + 1), :],
                            ag_b_out[r, 0].rearrange("(k p) c -> p k c", p=P),
                        )
                        nc.sync.dma_start(
                            cmp_b_lo[:, 4 * r : 4 * (r + 1), :],
                            ag_b_out[r, 1].rearrange("(k p) c -> p k c", p=P),
                        )
                    for hh in range(hA, h):
                        nc.gpsimd.tensor_copy(
                            wh4[:, hh, :, 0:dh],
                            cmp_b_hi[:, :, (hh - hA) * dh : (hh - hA + 1) * dh],
                        )
                        nc.vector.tensor_copy(
                            wl4[:, hh, :, 0:dh],
                            cmp_b_lo[:, :, (hh - hA) * dh : (hh - hA + 1) * dh],
                        )

                if debug_taps and li == 1:
                    nc.sync.dma_start(dbg_d[:], d_sb[:])
                    dbgw = small.tile([P, 8 * 33], dt.float32, tag="o_sb")
                    nc.vector.tensor_copy(dbgw[:], whrow[:, 7, :])
                    nc.sync.dma_start(dbg_wh[:], dbgw[:])

                # ---- (F) attention in head groups
                G = min(4, hA) if h > 1 else 1
                xnext = xTown_pool.tile([fout, ROWS], dt.float32, tag="xTown")
                for g0 in range(0, h, G):
                    gs = list(range(g0, min(g0 + G, h)))
                    ng = len(gs)
                    att_acc = []
                    for _k in gs:
                        att_t = attps.tile([33, ROWS], dt.float32, tag="att")
                        att_acc.append(att_t)
                    for jt in range(JT):
                        l_jt = work.tile([P, ng * ROWS], dt.float32, tag="l_jt")
                        for k, hh in enumerate(gs):
                            nc.vector._custom_dve(
                                LEAKY_BIAS_ADDMASK,
                                out=l_jt[:, k * ROWS : (k + 1) * ROWS],
                                in0=sreps[hh][:],
                                in1=maskT[:, jt, :],
                                s0=d_sb[:, jt, hh : hh + 1],
                                s1=ALPHA,
                            )
                        p_jt = work.tile([P, ng * ROWS], dt.bfloat16, tag="p_jt")
                        nc.scalar.activation(p_jt[:], l_jt[:], AF.Exp)
                        for k, hh in enumerate(gs):
                            nc.tensor.matmul(
                                att_acc[k][:],
                                whrow[:, jt]
                                .rearrange("p (a b) -> p a b", a=h)[:, hh, :],
                                p_jt[:, k * ROWS : (k + 1) * ROWS],
                                start=(jt == 0),
                                stop=False,
                            )
                            nc.tensor.matmul(
                                att_acc[k][:],
                                whrow_lo[:, jt]
                                .rearrange("p (a b) -> p a b", a=h)[:, hh, :],
                                p_jt[:, k * ROWS : (k + 1) * ROWS],
                                start=False,
                                stop=(jt == JT - 1),
                            )
                    # epilogue per head
                    for k, hh in enumerate(gs):
                        o_sb = small.tile([33, ROWS], dt.float32, tag="o_sb")
                        nc.scalar.copy(o_sb[:], att_acc[k][:])
                        r_sb = small.tile([1, ROWS], dt.float32, tag="vec1")
                        nc.vector.reciprocal(r_sb[:], o_sb[32:33, :])
                        rrep = small.tile([dh, ROWS], dt.float32, tag="rrep")
                        nc.gpsimd.partition_broadcast(rrep[:], r_sb[:])
                        ohead = small.tile([dh, ROWS], dt.float32, tag="ohead")
                        nc.gpsimd.tensor_mul(ohead[:], o_sb[0:dh, :], rrep[:])
                        if elu:
                            # elu(x) = max(x,0) - 1 + exp(min(x,0))
                            mmin = small.tile([dh, ROWS], dt.float32, tag="tmp1")
                            nc.gpsimd.tensor_scalar(
                                mmin[:], ohead[:], 0.0, None, mybir.AluOpType.min
                            )
                            emin = small.tile([dh, ROWS], dt.float32, tag="tmp2")
                            nc.scalar.activation(emin[:], mmin[:], AF.Exp)
                            rmax = small.tile([dh, ROWS], dt.float32, tag="tmp1")
                            nc.gpsimd.tensor_scalar(
                                rmax[:],
                                ohead[:],
                                0.0,
                                -1.0,
                                mybir.AluOpType.max,
                                mybir.AluOpType.add,
                            )
                            nc.gpsimd.tensor_add(ohead[:], rmax[:], emin[:])
                        nc.sync.dma_start(
                            xnext[hh * dh : (hh + 1) * dh, :], ohead[:]
                        )

                if debug_taps:
                    nc.sync.dma_start(dbg_x[li][:], xnext[:])
                if is_last:
                    psum_final = small.tile([fout, 1], dt.float32, tag="vec1")
                    nc.vector.reduce_sum(
                        psum_final[:], xnext[:], axis=mybir.AxisListType.X
                    )
                    nc.sync.dma_start(pool_out[:], psum_final[:])
                else:
                    xTown_cur = [xnext]

    nc.finalize()
    return nc


_NC_CACHE = None
_last_in_maps = None


def kernel(**inputs):
    global _NC_CACHE
    node_features = np.asarray(inputs["node_features"], dtype=np.float32)
    adj = np.ascontiguousarray(np.asarray(inputs["adj_mat"], dtype=np.int32))
    fc_w = np.asarray(inputs["fc_w"], dtype=np.float32)
    fc_b = np.asarray(inputs["fc_b"], dtype=np.float32)

    x0T = node_features.T  # [256, N]

    wext = {}
    ws = {}
    for li, (fin, fout, h, concat, _elu) in enumerate(CFG, start=1):
        dh = fout // h if concat else fout
        W = np.asarray(inputs[f"W{li}"], dtype=np.float32)  # [h, fin, dh]
        a_src = np.asarray(inputs[f"a_src{li}"], dtype=np.float32)  # [h, dh]
        a_dst = np.asarray(inputs[f"a_dst{li}"], dtype=np.float32)
        wcat = W.transpose(1, 0, 2).reshape(fin, h * dh)
        wd = np.einsum("hfd,hd->fh", W, a_dst).astype(np.float32)
        wsrc = np.einsum("hfd,hd->fh", W, a_src).astype(np.float32)
        import os as _os2
        hA = (h // 2 if h > 1 else 1) if not _os2.environ.get("NOSPLIT") else h
        wext[li] = np.ascontiguousarray(
            np.concatenate([wcat[:, : hA * dh], wd, wcat[:, hA * dh :]], axis=1)
        )
        ws[li] = np.ascontiguousarray(wsrc)

    in_maps = []
    for c in range(NCORE):
        m = {
            "adjrows": np.ascontiguousarray(adj[c * ROWS : (c + 1) * ROWS, :]),
            "x0T_own": np.ascontiguousarray(x0T[:, c * ROWS : (c + 1) * ROWS]),
        }
        for li in range(1, 6):
            m[f"wext{li}"] = wext[li]
            m[f"ws{li}"] = ws[li]
        in_maps.append(m)

    if _NC_CACHE is None:
        _NC_CACHE = build_kernel()
    nc = _NC_CACHE
    global _last_in_maps
    _last_in_maps = in_maps

    res = run_bass_kernel_spmd(nc, in_maps, list(range(NCORE)))
    total = np.zeros((8,), dtype=np.float32)
    for c in range(NCORE):
        total += res.results[c]["pool_part"][:, 0]
    pooled = total / np.float32(N)
    out = pooled @ fc_w + fc_b
    return out.astype(np.float32)



# revision 16
# speedup vs baseline: 7.2756x; 7.2756x over previous
"""GAT (5-layer, dense-adjacency) Trainium2 kernel, sharded across 8 NeuronCores.

v2 design:
- Query rows split 512/core; additive attention mask is precomputed on the
  host (transposed, f32) and DMA'd straight into SBUF.
- Layer 1 computes the FULL Wh from the replicated input features (no
  collective); layers 2-5 compute the own row-block of Wh and do ONE bf16
  AllGather per layer (no hi/lo split).
- Wh travels in a (dh+1)-strided layout with a ones column per head baked in
  before the gather, so softmax denominators fall out of the same matmul and
  no unpack/padding pass is needed: the gather lands directly in the
  attention operand layout.
- Engine specialization: DVE runs the fused leaky(s+d)+mask custom op (and
  reciprocals), ACT only exponentials, Pool all copies/broadcasts/epilogue
  math, PE all matmuls.
"""

import os as _os

import numpy as np

import concourse.bacc as bacc
import concourse.mybir as mybir
import concourse.tile as tile
from concourse.bass_utils import run_bass_kernel_spmd

import concourse.dve_ops as dve_ops
from concourse.dve_spec import Spec, Src0, Src1, C0, C1, maxx, lower
from concourse.dve_spec import _has_src1 as _spec_has_src1
from concourse.dve_uop import DveOpSpec

try:
    import ml_dtypes

    _BF16 = ml_dtypes.bfloat16
except ImportError:  # pragma: no cover
    _BF16 = np.float32

dt = mybir.dt
AF = mybir.ActivationFunctionType

# ---------------------------------------------------------------- constants
N = 4096
NCORE = 8
ROWS = N // NCORE  # 512 query rows per core
P = 128
JT = N // P  # 32 j-tiles
NEG = -30000.0  # additive mask for non-edges; exp(x-30000) == 0 in f32
ALPHA = 0.1
# (fin, fout, heads, concat, elu_after)
CFG = [
    (256, 128, 8, True, True),
    (128, 64, 8, True, True),
    (64, 32, 4, True, True),
    (32, 16, 1, True, False),
    (16, 8, 1, False, False),
]

# ---------------------------------------------------------------- custom op
LEAKY_BIAS_ADDMASK = dve_ops.DveOp(
    "LEAKY_BIAS_ADDMASK",
    Spec(
        body=maxx(Src0 + C0, (Src0 + C0) * C1) + Src1,
        reference=lambda in0, in1, s0, s1, imm2: (
            np.maximum(in0 + s0, (in0 + s0) * s1) + in1
        ).astype(np.float32),
    ),
    subdim=False,
    uops_sha={},
)


def _register_custom_op(op):
    if op.name in dve_ops._SUB_OPCODE_FOR_NAME:
        return
    idx = dve_ops._CUSTOM_DVE_ROW_BASE + len(dve_ops.OPS)
    assert idx < 0x20
    dve_ops.OPS.append(op)
    dve_ops.CUSTOM_DVE_SPECS[op.name] = op.spec
    dve_ops._SUB_OPCODE_FOR_NAME[op.name] = idx
    shas = {}
    for ver in ("v3", "v4"):
        try:
            s = DveOpSpec(
                name=op.name,
                opcode=idx,
                uops=lower(op.spec, ver=ver),
                rd1_en=_spec_has_src1(op.spec),
            )
            shas[ver] = s.sha(ver)
        except Exception:
            pass
    object.__setattr__(op, "uops_sha", shas)


_register_custom_op(LEAKY_BIAS_ADDMASK)


def _cwt(fin, fout, h, concat):
    dh = fout // h if concat else fout
    return h * (dh + 1) + h  # per-head (dh values + ones col) + h d-columns


# ---------------------------------------------------------------- builder
def build_kernel():
    debug_taps = bool(_os.environ.get("DEBUG_TAPS"))
    nc = bacc.Bacc("TRN2", target_bir_lowering=False, debug=False)

    maskd = nc.dram_tensor("maskd", [N, ROWS], dt.float32, kind="ExternalInput")
    x0T_full_d = nc.dram_tensor("x0T_full", [256, N], dt.float32, kind="ExternalInput")
    x0T_own_d = nc.dram_tensor("x0T_own", [256, ROWS], dt.float32, kind="ExternalInput")
    wext_dram = {}
    ws_dram = {}
    for li, (fin, fout, h, concat, _elu) in enumerate(CFG, start=1):
        cwt = _cwt(fin, fout, h, concat)
        wext_dram[li] = nc.dram_tensor(
            f"wext{li}", [fin, cwt], dt.float32, kind="ExternalInput"
        )
        ws_dram[li] = nc.dram_tensor(f"ws{li}", [fin, h], dt.float32, kind="ExternalInput")

    pool_out = nc.dram_tensor("pool_part", [8, 1], dt.float32, kind="ExternalOutput")
    if debug_taps:
        dbg_x = {}
        for _li, (_f, _fo, _h, _c, _e) in enumerate(CFG, start=1):
            dbg_x[_li] = nc.dram_tensor(
                f"dbg_x{_li}", [_fo, ROWS], dt.float32, kind="ExternalOutput"
            )
        dbg_srep = nc.dram_tensor("dbg_srep", [P, 8, ROWS], dt.float32, kind="ExternalOutput")
        dbg_d = nc.dram_tensor("dbg_d", [P, JT, 8], dt.float32, kind="ExternalOutput")
        dbg_cmp1 = nc.dram_tensor("dbg_cmp1", [P, JT, 144], dt.bfloat16, kind="ExternalOutput")
        dbg_sall = nc.dram_tensor("dbg_sall", [8, ROWS], dt.float32, kind="ExternalOutput")
        dbg_sflat = nc.dram_tensor("dbg_sflat", [1, 8 * ROWS], dt.float32, kind="ExternalOutput")

    with tile.TileContext(nc) as tc:
        with (
            tc.tile_pool(name="persist", bufs=1) as persist,
            tc.tile_pool(name="dram", bufs=1, space="DRAM") as drampool,
            tc.tile_pool(name="xTown", bufs=2) as xTown_pool,
            tc.tile_pool(name="ownsb", bufs=2) as ownsb_pool,
            tc.tile_pool(name="work", bufs=3) as work,
            tc.tile_pool(name="pwork", bufs=3) as pwork,
            tc.tile_pool(name="small", bufs=2) as small,
            tc.tile_pool(name="whps", bufs=2, space="PSUM") as whps,
            tc.tile_pool(name="sps", bufs=1, space="PSUM") as sps,
            tc.tile_pool(name="attps", bufs=5, space="PSUM") as attps,
        ):
            # ---------------- persistent tiles / input DMAs
            x0To = []
            for ft in range(2):
                xo = persist.tile(
                    [P, ROWS], dt.float32, tag=f"x0To{ft}", name=f"x0To{ft}"
                )
                nc.sync.dma_start(xo[:], x0T_own_d[ft * P : (ft + 1) * P, :])
                x0To.append(xo)

            wext_sb = {}
            ws_sb = {}
            for li, (fin, fout, h, concat, _elu) in enumerate(CFG, start=1):
                cwt = _cwt(fin, fout, h, concat)
                nft = (fin + P - 1) // P
                wext_sb[li] = []
                ws_sb[li] = []
                for ft in range(nft):
                    fr = min(P, fin - ft * P)
                    wt = persist.tile([fr, cwt], dt.float32, tag=f"wext{li}_{ft}")
                    nc.sync.dma_start(wt[:], wext_dram[li][ft * P : ft * P + fr, :])
                    wext_sb[li].append(wt)
                    st = persist.tile([fr, h], dt.float32, tag=f"ws{li}_{ft}")
                    nc.sync.dma_start(st[:], ws_dram[li][ft * P : ft * P + fr, :])
                    ws_sb[li].append(st)

            # mask: 4 chunked DMAs for finer-grained readiness
            maskT = persist.tile([P, JT, ROWS], dt.float32, tag="maskT")
            for q in range(4):
                nc.sync.dma_start(
                    maskT[:, q * 8 : (q + 1) * 8, :],
                    maskd[q * 8 * P : (q + 1) * 8 * P, :].rearrange(
                        "(jt p) i -> p jt i", p=P
                    ),
                )

            # per-layer gathered Wh (attention operand layout)
            cmp_sb = {}
            for li, (fin, fout, h, concat, _elu) in enumerate(CFG, start=1):
                cwt = _cwt(fin, fout, h, concat)
                cmp_sb[li] = persist.tile(
                    [P, JT, cwt], dt.bfloat16, tag=f"cmp{li}", name=f"cmp{li}"
                )

            srepT = persist.tile([P, 8, ROWS], dt.float32, tag="srepT")
            d_sb = persist.tile([P, JT, 8], dt.float32, tag="d_sb")
            s_all_sb = persist.tile([8, ROWS], dt.float32, tag="s_all_sb")
            s_flat = persist.tile([1, 8 * ROWS], dt.float32, tag="s_flat")

            x_cur = x0To

            for li, (fin, fout, h, concat, elu) in enumerate(CFG, start=1):
                dh = fout // h if concat else fout
                cwt = _cwt(fin, fout, h, concat)
                hde = h * (dh + 1)
                nft = (fin + P - 1) // P
                is_last = li == len(CFG)
                cmp = cmp_sb[li]

                # ---- (A) Wh in gathered layout
                if li == 1:
                    # full Wh from the replicated input (streamed); no collective
                    for k in range(JT):
                        pw = whps.tile([P, cwt], dt.float32, tag="pw")
                        for ft in range(nft):
                            xk = ownsb_pool.tile([P, P], dt.float32, tag="xk")
                            nc.sync.dma_start(
                                xk[:],
                                x0T_full_d[ft * P : (ft + 1) * P, k * P : (k + 1) * P],
                            )
                            nc.tensor.matmul(
                                pw[:],
                                xk[:],
                                wext_sb[li][ft][:],
                                start=(ft == 0),
                                stop=(ft == nft - 1),
                            )
                        nc.scalar.copy(cmp[:, k, :], pw[:])
                    for hh in range(h):
                        nc.gpsimd.memset(
                            cmp[:, :, hh * (dh + 1) + dh : hh * (dh + 1) + dh + 1], 1.0
                        )
                else:
                    own_sb = ownsb_pool.tile([P, 4, cwt], dt.bfloat16, tag="own_sb")
                    for k in range(4):
                        pw = whps.tile([P, cwt], dt.float32, tag="pw")
                        for ft in range(nft):
                            fr = min(P, fin - ft * P)
                            nc.tensor.matmul(
                                pw[:],
                                x_cur[ft][0:fr, k * P : (k + 1) * P],
                                wext_sb[li][ft][:],
                                start=(ft == 0),
                                stop=(ft == nft - 1),
                            )
                        nc.scalar.copy(own_sb[:, k, :], pw[:])
                    for hh in range(h):
                        nc.gpsimd.memset(
                            own_sb[:, :, hh * (dh + 1) + dh : hh * (dh + 1) + dh + 1],
                            1.0,
                        )
                    ag_in = drampool.tile([4 * P, cwt], dt.bfloat16, tag=f"agin{li}")
                    ag_out = drampool.tile(
                        [NCORE, 4 * P, cwt],
                        dt.bfloat16,
                        tag=f"agout{li}",
                        addr_space="Shared",
                    )
                    nc.sync.dma_start(
                        ag_in.rearrange("(k p) c -> p k c", p=P), own_sb[:]
                    )
                    nc.gpsimd.collective_compute(
                        "AllGather",
                        mybir.AluOpType.bypass,
                        replica_groups=[list(range(NCORE))],
                        ins=[ag_in.opt()],
                        outs=[ag_out.opt()],
                    )
                    nc.sync.dma_start(
                        cmp[:],
                        ag_out.rearrange("r (k p) c -> p (r k) c", p=P),
                    )

                # d columns (bf16-rounded; well within error budget)
                nc.vector.tensor_copy(d_sb[:, :, 0:h], cmp[:, :, hde:cwt])

                # ---- (B) s for own rows, then per-head partition broadcast
                ps_all = sps.tile([h, ROWS], dt.float32, tag="ps_all")
                for ft in range(nft):
                    fr = min(P, fin - ft * P)
                    nc.tensor.matmul(
                        ps_all[:],
                        ws_sb[li][ft][:],
                        x_cur[ft][0:fr, :],
                        start=(ft == 0),
                        stop=(ft == nft - 1),
                    )
                nc.scalar.copy(s_all_sb[0:h, :], ps_all[:])
                for hh in range(h):
                    nc.sync.dma_start(
                        s_flat[0:1, hh * ROWS : (hh + 1) * ROWS],
                        s_all_sb[hh : hh + 1, :],
                    )
                nc.gpsimd.partition_broadcast(
                    srepT[:, 0:h, :], s_flat[0:1, 0 : h * ROWS]
                )

                # ---- (C) attention
                xnext = xTown_pool.tile([fout, ROWS], dt.float32, tag="xTown")
                G = 4 if h >= 4 else h
                for g0 in range(0, h, G):
                    gs = list(range(g0, min(g0 + G, h)))
                    ng = len(gs)
                    att_acc = []
                    for _k in gs:
                        att_t = attps.tile([dh + 1, ROWS], dt.float32, tag="att")
                        att_acc.append(att_t)
                    for jt in range(JT):
                        l_jt = work.tile([P, ng * ROWS], dt.float32, tag="l_jt")
                        for k, hh in enumerate(gs):
                            nc.vector._custom_dve(
                                LEAKY_BIAS_ADDMASK,
                                out=l_jt[:, k * ROWS : (k + 1) * ROWS],
                                in0=srepT[:, hh, :],
                                in1=maskT[:, jt, :],
                                s0=d_sb[:, jt, hh : hh + 1],
                                s1=ALPHA,
                            )
                        p_jt = pwork.tile([P, ng * ROWS], dt.bfloat16, tag="p_jt")
                        nc.scalar.activation(p_jt[:], l_jt[:], AF.Exp)
                        for k, hh in enumerate(gs):
                            nc.tensor.matmul(
                                att_acc[k][:],
                                cmp[:, jt, hh * (dh + 1) : (hh + 1) * (dh + 1)],
                                p_jt[:, k * ROWS : (k + 1) * ROWS],
                                start=(jt == 0),
                                stop=(jt == JT - 1),
                            )
                    # epilogue per head
                    for k, hh in enumerate(gs):
                        o_sb = small.tile([dh + 1, ROWS], dt.float32, tag="o_sb")
                        nc.scalar.copy(o_sb[:], att_acc[k][:])
                        den0 = small.tile([1, ROWS], dt.float32, tag="den0")
                        nc.sync.dma_start(den0[:], o_sb[dh : dh + 1, :])
                        r_sb = small.tile([1, ROWS], dt.float32, tag="vec1")
                        nc.vector.reciprocal(r_sb[:], den0[:])
                        rrep = small.tile([dh, ROWS], dt.float32, tag="rrep")
                        nc.gpsimd.partition_broadcast(rrep[:], r_sb[:])
                        ohead = small.tile([dh, ROWS], dt.float32, tag="ohead")
                        nc.gpsimd.tensor_mul(ohead[:], o_sb[0:dh, :], rrep[:])
                        if elu:
                            # elu(x) = max(x,0) - 1 + exp(min(x,0))
                            mmin = small.tile([dh, ROWS], dt.float32, tag="tmp1")
                            nc.gpsimd.tensor_scalar(
                                mmin[:], ohead[:], 0.0, None, mybir.AluOpType.min
                            )
                            emin = small.tile([dh, ROWS], dt.float32, tag="tmp2")
                            nc.scalar.activation(emin[:], mmin[:], AF.Exp)
                            rmax = small.tile([dh, ROWS], dt.float32, tag="tmp1")
                            nc.gpsimd.tensor_scalar(
                                rmax[:],
                                ohead[:],
                                0.0,
                                -1.0,
                                mybir.AluOpType.max,
                                mybir.AluOpType.add,
                            )
                            nc.gpsimd.tensor_add(ohead[:], rmax[:], emin[:])
                        nc.sync.dma_start(
                            xnext[hh * dh : (hh + 1) * dh, :], ohead[:]
                        )

                if debug_taps:
                    nc.sync.dma_start(dbg_x[li][:], xnext[:])
                    if li == 1:
                        nc.sync.dma_start(dbg_srep[:], srepT[:])
                        nc.sync.dma_start(dbg_sall[:], s_all_sb[:])
                        nc.sync.dma_start(dbg_sflat[:], s_flat[:])
                        nc.sync.dma_start(dbg_d[:], d_sb[:])
                        nc.sync.dma_start(dbg_cmp1[:], cmp[:])
                if is_last:
                    psum_final = small.tile([fout, 1], dt.float32, tag="vec1")
                    nc.vector.reduce_sum(
                        psum_final[:], xnext[:], axis=mybir.AxisListType.X
                    )
                    nc.sync.dma_start(pool_out[:], psum_final[:])
                else:
                    x_cur = [xnext]

    nc.finalize()
    return nc


_NC_CACHE = None
_last_in_maps = None


def kernel(**inputs):
    global _NC_CACHE, _last_in_maps
    node_features = np.asarray(inputs["node_features"], dtype=np.float32)
    adj = np.asarray(inputs["adj_mat"], dtype=np.int32)
    fc_w = np.asarray(inputs["fc_w"], dtype=np.float32)
    fc_b = np.asarray(inputs["fc_b"], dtype=np.float32)

    x0T = np.ascontiguousarray(node_features.T)  # [256, N]
    # additive mask, transposed: maskd[j, i_own] = 0 if adj[i,j]>0 else -30000
    adjT = adj.T  # [j, i]
    mask_add_T = np.where(adjT > 0, np.float32(0.0), np.float32(NEG))

    wext = {}
    ws = {}
    for li, (fin, fout, h, concat, _elu) in enumerate(CFG, start=1):
        dh = fout // h if concat else fout
        W = np.asarray(inputs[f"W{li}"], dtype=np.float32)  # [h, fin, dh]
        a_src = np.asarray(inputs[f"a_src{li}"], dtype=np.float32)  # [h, dh]
        a_dst = np.asarray(inputs[f"a_dst{li}"], dtype=np.float32)
        wd = np.einsum("hfd,hd->fh", W, a_dst).astype(np.float32)
        wsrc = np.einsum("hfd,hd->fh", W, a_src).astype(np.float32)
        cwt = _cwt(fin, fout, h, concat)
        we = np.zeros((fin, cwt), dtype=np.float32)
        for hh in range(h):
            we[:, hh * (dh + 1) : hh * (dh + 1) + dh] = W[hh]
        we[:, h * (dh + 1) :] = wd
        wext[li] = np.ascontiguousarray(we)
        ws[li] = np.ascontiguousarray(wsrc)

    in_maps = []
    for c in range(NCORE):
        m = {
            "maskd": np.ascontiguousarray(
                mask_add_T[:, c * ROWS : (c + 1) * ROWS]
            ),
            "x0T_full": x0T,
            "x0T_own": np.ascontiguousarray(x0T[:, c * ROWS : (c + 1) * ROWS]),
        }
        for li in range(1, 6):
            m[f"wext{li}"] = wext[li]
            m[f"ws{li}"] = ws[li]
        in_maps.append(m)

    if _NC_CACHE is None:
        _NC_CACHE = build_kernel()
    nc = _NC_CACHE
    _last_in_maps = in_maps

    res = run_bass_kernel_spmd(nc, in_maps, list(range(NCORE)))
    total = np.zeros((8,), dtype=np.float32)
    for c in range(NCORE):
        total += res.results[c]["pool_part"][:, 0]
    pooled = total / np.float32(N)
    out = pooled @ fc_w + fc_b
    return out.astype(np.float32)
